# revision 16
# baseline (speedup 1.0000x reference)
"""Causal multi-head attention on 8 TRN2 NeuronCores — v2.

Sharding: core c = (batch b=c//2, head-group g=c%2). Each core computes QKV
projections + causal attention for its 8 heads over the full sequence of its
batch; pairwise AllGathers exchange attention outputs; each core runs the
output projection for its half of the output columns (per-core host data
keeps the program uniform across cores).

v2 scheduling changes vs v1 (same math, bf16 everywhere):
  - input DMAs split across the Sync and Scalar queues, ordered by first
    use; q/k weights in per-pair host layout so each pair loads with 2KB
    lines just before it is needed.
  - v_aug ones/zero pattern comes from memsets instead of a DMA'd pattern
    (ones col 0 -> softmax denominator in psum row 0, values rows 64:128;
    reciprocal_approx_fast and partition_broadcast require base 0).
  - normalization is eager at the end of each chunk and reads the av psum
    directly (no bounce copies).
  - output projection is spread across the attention pipeline as PE
    fillers: A1{0,4}@p1, A2{1,5}@p2, A3{2,6}@p3, then B{3,7} per q-tile.
    Pair 3's attention is gathered in two pieces (cols 0:1536 after j=2,
    cols 1536:2048 after j=3) so only 4 q-tiles of work remain after the
    last (small) AllGather.
  - one AllGather per pair (absorbs inter-core skew earlier).
"""

import numpy as np
import ml_dtypes

import concourse.bass as bass
import concourse.mybir as mybir
import concourse.tile as tile
from concourse import bacc
from concourse import bass_utils

BF16 = mybir.dt.bfloat16
FP16 = mybir.dt.float16
F32 = mybir.dt.float32

B, S, D = 4, 2048, 1024
H, DK = 16, 64
HPG = 8          # heads per group (per core)
DG = HPG * DK    # 512, d-range per core
NPAIR = 4        # head pairs per core
SC = 512         # sequence chunk (matmul free dim)
NSC = S // SC    # 4
KB = 128         # key block
NKB = S // KB    # 16
P = 128
NI = D // P      # 8

_cache = {}


def _build():
    nc = bacc.Bacc("TRN2", target_bir_lowering=False, debug=False, num_devices=8)

    xT = nc.dram_tensor("xT", [D, S], BF16, kind="ExternalInput")
    wqH = nc.dram_tensor("wqH", [NPAIR, P, NI, P], BF16, kind="ExternalInput")
    wkH = nc.dram_tensor("wkH", [NPAIR, P, NI, P], BF16, kind="ExternalInput")
    wvT = nc.dram_tensor("wvT", [D, DG], BF16, kind="ExternalInput")
    woT = nc.dram_tensor("woT", [D, DG], BF16, kind="ExternalInput")
    bq = nc.dram_tensor("bq", [P, NPAIR], F32, kind="ExternalInput")
    bk = nc.dram_tensor("bk", [P, NPAIR], F32, kind="ExternalInput")
    bv_bc = nc.dram_tensor("bv_bc", [P, DG], F32, kind="ExternalInput")
    bo_bc = nc.dram_tensor("bo_bc", [P, DG], F32, kind="ExternalInput")
    masks = nc.dram_tensor("masks", [4, P, SC], BF16, kind="ExternalInput")
    out = nc.dram_tensor("out", [S, DG], F32, kind="ExternalOutput")

    with tile.TileContext(nc) as tc:
        _emit(nc, tc, xT, wqH, wkH, wvT, woT, bq, bk, bv_bc, bo_bc, masks, out)
    nc.compile()
    return nc


def _emit(nc, tc, xT, wqH, wkH, wvT, woT, bq, bk, bv_bc, bo_bc, masks, out):
    ctxs = []

    def pool(name, bufs, space="SBUF"):
        cm = tc.tile_pool(name=name, bufs=bufs, space=space)
        p = cm.__enter__()
        ctxs.append(cm)
        return p

    const = pool("const", 1)
    dram = pool("dram", 1, space="DRAM")
    qk_pool = pool("qk", 2)
    att_pool = pool("att", 2)
    exp_pool = pool("exp", 5)
    small = pool("small", 2)
    out_pool = pool("outp", 2)
    ps_qk = pool("ps_qk", 2, space="PSUM")
    ps_sc = pool("ps_sc", 2, space="PSUM")
    ps_av = pool("ps_av", 2, space="PSUM")

    # ---- constant tiles ----
    xt = const.tile([P, NI, S], BF16, name="xt")
    wq = const.tile([P, NPAIR, NI, P], BF16, name="wq")
    wk = const.tile([P, NPAIR, NI, P], BF16, name="wk")
    wv = const.tile([P, NI, DG], BF16, name="wv")
    wo = const.tile([P, NI, DG], BF16, name="wo")
    bq_t = const.tile([P, NPAIR], F32, name="bq_t")
    bk_t = const.tile([P, NPAIR], F32, name="bk_t")
    bv_t = const.tile([P, DG], F32, name="bv_t")
    bo_t = const.tile([P, DG], F32, name="bo_t")
    mask_t = const.tile([P, 4, SC], BF16, name="mask_t")
    # v_aug[t]: [128, 8, 128]; per head h: col 0 = ones (softmax denominator
    # row in psum partition 0), cols 1:64 = zeros (partition alignment pad),
    # cols 64:128 = v.
    v_aug = [const.tile([P, HPG * P], BF16, name=f"va{t}", tag=f"va{t}")
             for t in range(NKB)]

    # ---- input DMA preloads, ordered by first use, split across queues ----
    # scalar queue: weights path (scalar engine is idle until first exp)
    nc.scalar.dma_start(wq[:, 0], wqH[0])
    nc.scalar.dma_start(wk[:, 0], wkH[0])
    nc.scalar.dma_start(bq_t[:], bq[:])
    nc.scalar.dma_start(bk_t[:], bk[:])
    for i in range(NI):
        nc.scalar.dma_start(wv[:, i, :], wvT[P * i:P * (i + 1), :])
    nc.scalar.dma_start(bv_t[:], bv_bc[:])
    for r in range(4):
        nc.scalar.dma_start(mask_t[:, r, :], masks[r])
    for p in range(1, NPAIR):
        nc.scalar.dma_start(wq[:, p], wqH[p])
        nc.scalar.dma_start(wk[:, p], wkH[p])
    # sync queue: x, then the out-projection weights
    for i in range(NI):
        nc.sync.dma_start(xt[:, i, 0:SC], xT[P * i:P * (i + 1), 0:SC])
    for c in range(1, NSC):
        for i in range(NI):
            nc.sync.dma_start(xt[:, i, SC * c:SC * (c + 1)],
                              xT[P * i:P * (i + 1), SC * c:SC * (c + 1)])
    for i in range(NI):
        nc.sync.dma_start(wo[:, i, :], woT[P * i:P * (i + 1), :])
    nc.sync.dma_start(bo_t[:], bo_bc[:])
    # ones/zeros columns of v_aug (gpsimd, cheap)
    for t in range(NKB):
        va3 = v_aug[t].rearrange("p (h c) -> p h c", c=P)
        nc.gpsimd.memset(va3[:, :, 0:DK], 0.0)
        nc.gpsimd.memset(va3[:, :, 0:1], 1.0)

    # DRAM bounce buffers for the AllGathers
    agin = [dram.tile([P, S], BF16, name=f"agin{p}") for p in range(3)]
    agout = [dram.tile([2, P, S], BF16, name=f"agout{p}") for p in range(3)]
    agin3a = dram.tile([P, 3 * SC], BF16, name="agin3a")
    agin3b = dram.tile([P, SC], BF16, name="agin3b")
    agout3a = dram.tile([2, P, 3 * SC], BF16, name="agout3a")
    agout3b = dram.tile([2, P, SC], BF16, name="agout3b")

    groups = [[0, 1], [2, 3], [4, 5], [6, 7]]

    qT_pair = [qk_pool.tile([P, S], BF16, tag="qT", name=f"qTp{pp}")
               for pp in range(NPAIR)]
    kT_pair = [qk_pool.tile([P, S], BF16, tag="kT", name=f"kTp{pp}")
               for pp in range(NPAIR)]

    # gathered attention outputs (out-proj lhsT), split 1536/512 in q to
    # match pair 3's two gathers
    agt_a = [const.tile([P, 3 * SC], BF16, name=f"agta{i}", tag=f"agta{i}")
             for i in range(NI)]
    agt_b = [const.tile([P, SC], BF16, name=f"agtb{i}", tag=f"agtb{i}")
             for i in range(NI)]
    # out-proj partials (blocks 0,1,2,4,5,6 accumulated via fp16)
    part_lo = const.tile([P, NI, SC], FP16, name="part_lo")
    part_hi = const.tile([P, NI, SC], FP16, name="part_hi")

    def qk_chunk(p, sc):
        """q/k projections for pair p, seq chunk sc."""
        ssl = slice(SC * sc, SC * (sc + 1))
        ps_q = ps_qk.tile([P, SC], F32, tag="psqk", name=f"psq{p}_{sc}")
        for i in range(NI):
            nc.tensor.matmul(ps_q[:], lhsT=wq[:, p, i, :],
                             rhs=xt[:, i, ssl], start=(i == 0), stop=(i == 7))
        nc.vector.tensor_add(qT_pair[p][:, ssl], ps_q[:],
                             bq_t[:, p:p + 1].to_broadcast((P, SC)))
        ps_k = ps_qk.tile([P, SC], F32, tag="psqk", name=f"psk{p}_{sc}")
        for i in range(NI):
            nc.tensor.matmul(ps_k[:], lhsT=wk[:, p, i, :],
                             rhs=xt[:, i, ssl], start=(i == 0), stop=(i == 7))
        nc.vector.tensor_add(kT_pair[p][:, ssl], ps_k[:],
                             bk_t[:, p:p + 1].to_broadcast((P, SC)))

    def v_sub(t):
        """v projection for seq tile t (all 8 heads) into v_aug[t]."""
        ps_v = ps_qk.tile([P, DG], F32, tag="psqk", name=f"psv{t}")
        for i in range(NI):
            nc.tensor.matmul(ps_v[:], lhsT=xt[:, i, P * t:P * (t + 1)],
                             rhs=wv[:, i, :], start=(i == 0), stop=(i == 7))
        va3 = v_aug[t].rearrange("p (h c) -> p h c", c=P)
        nc.vector.tensor_add(va3[:, :, DK:P],
                             ps_v[:].rearrange("p (h c) -> p h c", c=DK),
                             bv_t[:].rearrange("p (h c) -> p h c", c=DK))

    def attention_chunk(p, j, att, fillers):
        """Causal attention for head pair p, q chunk j; eager normalize."""
        avs = [ps_av.tile([P, SC], F32, tag="av", name=f"av{p}_{j}_{h}")
               for h in range(2)]
        nkb = 4 * (j + 1)
        kbs = list(range(4 * j, nkb)) + list(range(0, 4 * j))  # diag first
        pending = []
        issued = [0]

        def issue_av(item):
            kb, qlo, et = item
            et3 = et.rearrange("p (h w) -> p h w", w=SC)
            for h in range(2):
                hh = 2 * p + h
                va3 = v_aug[kb].rearrange("p (h c) -> p h c", c=P)
                nc.tensor.matmul(avs[h][:, qlo:], lhsT=va3[:, hh, :],
                                 rhs=et3[:, h, qlo:],
                                 start=(issued[0] == 0),
                                 stop=(issued[0] == nkb - 1))
            issued[0] += 1

        for n, kb in enumerate(kbs):
            r = kb - 4 * j  # >= 0 on diagonal blocks
            qlo = P * r if r >= 0 else 0
            ps_s = ps_sc.tile([P, 2 * SC], F32, tag="sc", name=f"pss{p}_{j}_{kb}")
            for h in range(2):
                hb = slice(DK * h, DK * (h + 1))
                nc.tensor.matmul(
                    ps_s[:, SC * h + qlo:SC * (h + 1)],
                    lhsT=kT_pair[p][hb, P * kb:P * (kb + 1)],
                    rhs=qT_pair[p][hb, SC * j + qlo:SC * (j + 1)],
                    start=True, stop=True)
            et = exp_pool.tile([P, 2 * SC], BF16, tag="exp", name=f"et{p}_{j}_{kb}")
            ps3 = ps_s.rearrange("p (h w) -> p h w", w=SC)
            et3 = et.rearrange("p (h w) -> p h w", w=SC)
            nc.scalar.activation(et3[:, :, qlo:], ps3[:, :, qlo:],
                                 mybir.ActivationFunctionType.Exp, scale=0.125)
            if r >= 0:
                nc.vector.tensor_mul(
                    et3[:, :, qlo:], et3[:, :, qlo:],
                    mask_t[:, r:r + 1, qlo:].to_broadcast((P, 2, SC - qlo)))
            pending.append((kb, qlo, et))
            while len(pending) > 3:
                issue_av(pending.pop(0))
            if fillers:
                fillers.pop(0)()
        while pending:
            issue_av(pending.pop(0))

        # eager normalize straight from psum; denominator at partition 0
        for h in range(2):
            sums = small.tile([1, SC], F32, tag="sums", name=f"sums{p}_{j}_{h}")
            nc.vector.reciprocal_approx_fast(sums[0:1, :], avs[h][0:1, :])
            rb = small.tile([P, SC], F32, tag="rb", name=f"rb{p}_{j}_{h}")
            nc.gpsimd.partition_broadcast(rb[:], sums[0:1, :])
            nc.vector.tensor_mul(att[h][DK:P, SC * j:SC * (j + 1)],
                                 avs[h][DK:P, :], rb[DK:P, :])

    def agt_lhsT(i, qt):
        if qt < 12:
            return agt_a[i][:, P * qt:P * (qt + 1)]
        return agt_b[i][:, P * (qt - 12):P * (qt - 11)]

    def part_slice(qt):
        t = part_lo if qt < 8 else part_hi
        return t[:, qt % 8, :]

    def op_acc(qt, blocks, first):
        """Accumulate out-proj blocks into the fp16 partial for q-tile qt."""
        ps_o = ps_qk.tile([P, DG], F32, tag="psqk", name=f"pso{qt}_{blocks[0]}")
        for n, i in enumerate(blocks):
            nc.tensor.matmul(ps_o[:], lhsT=agt_lhsT(i, qt), rhs=wo[:, i, :],
                             start=(n == 0), stop=(n == len(blocks) - 1))
        t = part_slice(qt)
        nc.vector.tensor_add(t, ps_o[:], bo_t[:] if first else t)

    def op_final(qt):
        """Blocks 3,7 + partial -> out, DMA."""
        ps_o = ps_qk.tile([P, DG], F32, tag="psqk", name=f"psof{qt}")
        for n, i in enumerate([3, 7]):
            nc.tensor.matmul(ps_o[:], lhsT=agt_lhsT(i, qt), rhs=wo[:, i, :],
                             start=(n == 0), stop=(n == 1))
        ot = out_pool.tile([P, DG], F32, tag="ot", name=f"ot{qt}")
        nc.vector.tensor_add(ot[:], ps_o[:], part_slice(qt))
        nc.sync.dma_start(out[P * qt:P * (qt + 1), :], ot[:])

    def gather(in_ap, out_ap):
        nc.gpsimd.collective_compute(
            "AllGather", mybir.AluOpType.bypass, replica_groups=groups,
            ins=[in_ap.opt()], outs=[out_ap.opt()])

    # ---- prelude: pair-0 q/k and chunk-0 v ----
    for sc in range(NSC):
        qk_chunk(0, sc)
    for t in range(4):
        v_sub(t)

    # ---- per-chunk filler schedules ----
    def F(fn, *a):
        return lambda: fn(*a)

    fillers = {
        0: [[F(v_sub, t) for t in range(4, 8)],
            [F(v_sub, t) for t in range(8, 12)],
            [F(v_sub, t) for t in range(12, 16)] + [F(qk_chunk, 1, 0)],
            [F(qk_chunk, 1, sc) for sc in range(1, 4)]],
        1: [[F(qk_chunk, 2, 0)],
            [F(qk_chunk, 2, 1)],
            [F(qk_chunk, 2, 2)] + [F(op_acc, qt, [0, 4], True) for qt in range(6)],
            [F(qk_chunk, 2, 3)] + [F(op_acc, qt, [0, 4], True) for qt in range(6, 16)]],
        2: [[F(qk_chunk, 3, 0)],
            [F(qk_chunk, 3, 1)],
            [F(qk_chunk, 3, 2)] + [F(op_acc, qt, [1, 5], False) for qt in range(6)],
            [F(qk_chunk, 3, 3)] + [F(op_acc, qt, [1, 5], False) for qt in range(6, 16)]],
        3: [[],
            [],
            [F(op_acc, qt, [2, 6], False) for qt in range(8)],
            [F(op_acc, qt, [2, 6], False) for qt in range(8, 16)]],
    }

    # ---- attention pipeline ----
    for p in range(NPAIR):
        att = [att_pool.tile([P, S], BF16, tag=f"att{h}", name=f"att{p}_{h}")
               for h in range(2)]
        for j in range(NSC):
            ch = fillers[p][j]
            attention_chunk(p, j, att, ch)
            for f in ch:  # leftover fillers for this chunk
                f()
            ch.clear()
            if p == 3 and j == 2:
                # 3/4 gather for pair 3 (cols 0:1536)
                nc.gpsimd.dma_start(agin3a[0:DK, :], att[0][DK:P, 0:3 * SC])
                nc.gpsimd.dma_start(agin3a[DK:P, :], att[1][DK:P, 0:3 * SC])
                gather(agin3a[:], agout3a[:])
                for gs in range(2):
                    nc.sync.dma_start(agt_a[4 * gs + 3][:], agout3a[gs])
        if p < 3:
            nc.gpsimd.dma_start(agin[p][0:DK, :], att[0][DK:P, :])
            nc.gpsimd.dma_start(agin[p][DK:P, :], att[1][DK:P, :])
            gather(agin[p][:], agout[p][:])
            for gs in range(2):
                i = 4 * gs + p
                nc.sync.dma_start(agt_a[i][:], agout[p][gs][:, 0:3 * SC])
                nc.sync.dma_start(agt_b[i][:], agout[p][gs][:, 3 * SC:])
        else:
            # final 1/4 gather (cols 1536:2048)
            nc.gpsimd.dma_start(agin3b[0:DK, :], att[0][DK:P, 3 * SC:])
            nc.gpsimd.dma_start(agin3b[DK:P, :], att[1][DK:P, 3 * SC:])
            gather(agin3b[:], agout3b[:])
            for qt in range(12):       # B1: covered by the 3/4 gather
                op_final(qt)
            for gs in range(2):
                nc.sync.dma_start(agt_b[4 * gs + 3][:], agout3b[gs])
            for qt in range(12, 16):   # B2: after the final gather
                op_final(qt)

    for cm in reversed(ctxs):
        cm.__exit__(None, None, None)


def _prep_in_maps(x, Wq, bq, Wk, bk, Wv, bv, Wo, bo):
    bf16 = ml_dtypes.bfloat16
    in_maps = []
    mask = np.zeros((4, P, SC), dtype=bf16)
    for r in range(4):
        k_idx = np.arange(P)[:, None]
        q_idx = np.arange(SC)[None, :]
        mask[r] = (q_idx >= P * r + k_idx).astype(bf16)

    def pair_layout(Wg):
        # [p, r, i, c] with W[128p+c, 128i+r]
        return np.ascontiguousarray(
            Wg.reshape(NPAIR, P, NI, P).transpose(0, 3, 2, 1)).astype(bf16)

    for c in range(8):
        b, g = divmod(c, 2)
        dsl = slice(g * DG, (g + 1) * DG)
        in_maps.append({
            "xT": np.ascontiguousarray(x[b].T).astype(bf16),
            "wqH": pair_layout(Wq[dsl]),
            "wkH": pair_layout(Wk[dsl]),
            "wvT": np.ascontiguousarray(Wv[dsl].T).astype(bf16),
            "woT": np.ascontiguousarray(Wo[dsl].T).astype(bf16),
            "bq": np.ascontiguousarray(bq[dsl].reshape(NPAIR, P).T.astype(np.float32)),
            "bk": np.ascontiguousarray(bk[dsl].reshape(NPAIR, P).T.astype(np.float32)),
            "bv_bc": np.broadcast_to(bv[dsl].astype(np.float32), (P, DG)).copy(),
            "bo_bc": np.broadcast_to(bo[dsl].astype(np.float32), (P, DG)).copy(),
            "masks": mask,
        })
    return in_maps


def kernel(x, Wq, bq, Wk, bk, Wv, bv, Wo, bo, _trace=False, _trace_kwargs=None):
    x, Wq, bq, Wk, bk = map(np.asarray, (x, Wq, bq, Wk, bk))
    Wv, bv, Wo, bo = map(np.asarray, (Wv, bv, Wo, bo))
    if "nc" not in _cache:
        _cache["nc"] = _build()
    nc = _cache["nc"]
    in_maps = _prep_in_maps(x, Wq, bq, Wk, bk, Wv, bv, Wo, bo)
    res = bass_utils.run_bass_kernel_spmd(
        nc, in_maps, core_ids=list(range(8)), trace=_trace,
        **(_trace_kwargs or {}))
    _cache["last_result"] = res
    out = np.empty((B, S, D), dtype=np.float32)
    for c in range(8):
        b, g = divmod(c, 2)
        out[b, :, g * DG:(g + 1) * DG] = res.results[c]["out"]
    return out


# revision 30
# speedup vs baseline: 1.0602x; 1.0602x over previous
"""Causal multi-head attention on 8 TRN2 NeuronCores — v2.

Sharding: core c = (batch b=c//2, head-group g=c%2). Each core computes QKV
projections + causal attention for its 8 heads over the full sequence of its
batch; pairwise AllGathers exchange attention outputs; each core runs the
output projection for its half of the output columns (per-core host data
keeps the program uniform across cores).

v2 scheduling changes vs v1 (same math, bf16 everywhere):
  - input DMAs split across the Sync and Scalar queues, ordered by first
    use; q/k weights in per-pair host layout so each pair loads with 2KB
    lines just before it is needed.
  - v_aug ones/zero pattern comes from memsets instead of a DMA'd pattern
    (ones col 0 -> softmax denominator in psum row 0, values rows 64:128;
    reciprocal_approx_fast and partition_broadcast require base 0).
  - normalization is eager at the end of each chunk and reads the av psum
    directly (no bounce copies).
  - output projection is spread across the attention pipeline as PE
    fillers: A1{0,4}@p1, A2{1,5}@p2, A3{2,6}@p3, then B{3,7} per q-tile.
    Pair 3's attention is gathered in two pieces (cols 0:1536 after j=2,
    cols 1536:2048 after j=3) so only 4 q-tiles of work remain after the
    last (small) AllGather.
  - one AllGather per pair (absorbs inter-core skew earlier).
"""

import numpy as np
import ml_dtypes

import concourse.bass as bass
import concourse.mybir as mybir
import concourse.tile as tile
from concourse import bacc
from concourse import bass_utils

BF16 = mybir.dt.bfloat16
FP16 = mybir.dt.float16
F32 = mybir.dt.float32

B, S, D = 4, 2048, 1024
H, DK = 16, 64
HPG = 8          # heads per group (per core)
DG = HPG * DK    # 512, d-range per core
NPAIR = 4        # head pairs per core
SC = 512         # sequence chunk (matmul free dim)
NSC = S // SC    # 4
KB = 128         # key block
NKB = S // KB    # 16
P = 128
NI = D // P      # 8

_cache = {}


def _build():
    nc = bacc.Bacc("TRN2", target_bir_lowering=False, debug=False, num_devices=8)

    xT = nc.dram_tensor("xT", [D, S], BF16, kind="ExternalInput")
    wqH = nc.dram_tensor("wqH", [NPAIR, P, NI, P], BF16, kind="ExternalInput")
    wkH = nc.dram_tensor("wkH", [NPAIR, P, NI, P], BF16, kind="ExternalInput")
    wvT = nc.dram_tensor("wvT", [D, DG], BF16, kind="ExternalInput")
    woT = nc.dram_tensor("woT", [D, DG], BF16, kind="ExternalInput")
    bq = nc.dram_tensor("bq", [P, NPAIR], F32, kind="ExternalInput")
    bk = nc.dram_tensor("bk", [P, NPAIR], F32, kind="ExternalInput")
    bv_bc = nc.dram_tensor("bv_bc", [P, DG], F32, kind="ExternalInput")
    bo_bc = nc.dram_tensor("bo_bc", [P, DG], F32, kind="ExternalInput")
    masks = nc.dram_tensor("masks", [P, SC], BF16, kind="ExternalInput")
    out = nc.dram_tensor("out", [S, DG], F32, kind="ExternalOutput")

    with tile.TileContext(nc) as tc:
        _emit(nc, tc, xT, wqH, wkH, wvT, woT, bq, bk, bv_bc, bo_bc, masks, out)
    nc.compile()
    return nc


def _emit(nc, tc, xT, wqH, wkH, wvT, woT, bq, bk, bv_bc, bo_bc, masks, out):
    ctxs = []

    def pool(name, bufs, space="SBUF"):
        cm = tc.tile_pool(name=name, bufs=bufs, space=space)
        p = cm.__enter__()
        ctxs.append(cm)
        return p

    const = pool("const", 1)
    dram = pool("dram", 1, space="DRAM")
    qk_pool = pool("qk", 2)
    att_pool = pool("att", 2)
    exp_pool = pool("exp", 5)
    small = pool("small", 2)
    out_pool = pool("outp", 4)
    ps_qk = pool("ps_qk", 2, space="PSUM")
    ps_sc = pool("ps_sc", 2, space="PSUM")
    ps_av = pool("ps_av", 2, space="PSUM")

    # ---- constant tiles ----
    xt = const.tile([P, NI, S], BF16, name="xt")
    wq = const.tile([P, NPAIR, NI, P], BF16, name="wq")
    wk = const.tile([P, NPAIR, NI, P], BF16, name="wk")
    wv = const.tile([P, NI, DG], BF16, name="wv")
    wo = const.tile([P, NI, DG], BF16, name="wo")
    bq_t = const.tile([P, NPAIR], F32, name="bq_t")
    bk_t = const.tile([P, NPAIR], F32, name="bk_t")
    bv_t = const.tile([P, DG], F32, name="bv_t")
    bo_t = const.tile([P, DG], F32, name="bo_t")
    # single lower-triangle mask; mask for diag offset r is tri[:, 0, 0:SC-128r]
    tri = const.tile([P, 1, SC], BF16, name="tri")
    # v_aug[t]: [128, 8, 128]; per head h: col 0 = ones (softmax denominator
    # row in psum partition 0), cols 1:64 = zeros (partition alignment pad),
    # cols 64:128 = v.
    v_aug = [const.tile([P, HPG * P], BF16, name=f"va{t}", tag=f"va{t}")
             for t in range(NKB)]

    # ---- input DMA preloads, ordered by first use, split across queues ----
    # sync queue: pair-0 weights and x chunk 0 first (first matmul inputs)
    nc.sync.dma_start(wq[:, 0], wqH[0])
    nc.sync.dma_start(wk[:, 0], wkH[0])
    for i in range(NI):
        nc.sync.dma_start(xt[:, i, 0:SC], xT[P * i:P * (i + 1), 0:SC])
    # gpsimd: q/k biases (tiny, needed by the first DVE adds)
    nc.gpsimd.dma_start(bq_t[:], bq[:])
    nc.gpsimd.dma_start(bk_t[:], bk[:])
    # scalar queue: remaining weights (scalar engine is idle until first exp)
    nc.scalar.dma_start(wq[:, 1], wqH[1])
    nc.scalar.dma_start(wk[:, 1], wkH[1])
    for i in range(NI):
        nc.scalar.dma_start(wv[:, i, :], wvT[P * i:P * (i + 1), :])
    nc.scalar.dma_start(bv_t[:], bv_bc[:])
    nc.scalar.dma_start(tri[:, 0, :], masks[:])
    for p in range(2, NPAIR):
        nc.scalar.dma_start(wq[:, p], wqH[p])
        nc.scalar.dma_start(wk[:, p], wkH[p])
    # sync queue: rest of x, then the out-projection weights
    for c in range(1, NSC):
        for i in range(NI):
            nc.sync.dma_start(xt[:, i, SC * c:SC * (c + 1)],
                              xT[P * i:P * (i + 1), SC * c:SC * (c + 1)])
    for i in range(NI):
        nc.sync.dma_start(wo[:, i, :], woT[P * i:P * (i + 1), :])
    nc.sync.dma_start(bo_t[:], bo_bc[:])
    # ones/zeros columns of v_aug (gpsimd, cheap)
    for t in range(NKB):
        va3 = v_aug[t].rearrange("p (h c) -> p h c", c=P)
        nc.gpsimd.memset(va3[:, :, 0:DK], 0.0)
        nc.gpsimd.memset(va3[:, :, 0:1], 1.0)

    # DRAM bounce buffers for the AllGathers
    agin = [dram.tile([P, S], BF16, name=f"agin{p}") for p in range(3)]
    agout = [dram.tile([2, P, S], BF16, name=f"agout{p}") for p in range(3)]
    agin3a = dram.tile([P, 3 * SC], BF16, name="agin3a")
    agin3b = dram.tile([P, SC], BF16, name="agin3b")
    agout3a = dram.tile([2, P, 3 * SC], BF16, name="agout3a")
    agout3b = dram.tile([2, P, SC], BF16, name="agout3b")

    groups = [[0, 1], [2, 3], [4, 5], [6, 7]]

    qT_pair = [qk_pool.tile([P, S], BF16, tag="qT", name=f"qTp{pp}")
               for pp in range(NPAIR)]
    kT_pair = [qk_pool.tile([P, S], BF16, tag="kT", name=f"kTp{pp}")
               for pp in range(NPAIR)]

    # gathered attention outputs (out-proj lhsT), split 1536/512 in q to
    # match pair 3's two gathers
    agt_a = [const.tile([P, 3 * SC], BF16, name=f"agta{i}", tag=f"agta{i}")
             for i in range(NI)]
    agt_b = [const.tile([P, SC], BF16, name=f"agtb{i}", tag=f"agtb{i}")
             for i in range(NI)]
    # out-proj partials (blocks 0,1,2,4,5,6 accumulated via fp16)
    part_lo = const.tile([P, NI, SC], FP16, name="part_lo")
    part_hi = const.tile([P, NI, SC], FP16, name="part_hi")

    def qk_chunk(p, sc):
        """q/k projections for pair p, seq chunk sc."""
        ssl = slice(SC * sc, SC * (sc + 1))
        ps_q = ps_qk.tile([P, SC], F32, tag="psqk", name=f"psq{p}_{sc}")
        for i in range(NI):
            nc.tensor.matmul(ps_q[:], lhsT=wq[:, p, i, :],
                             rhs=xt[:, i, ssl], start=(i == 0), stop=(i == 7))
        nc.vector.tensor_add(qT_pair[p][:, ssl], ps_q[:],
                             bq_t[:, p:p + 1].to_broadcast((P, SC)))
        ps_k = ps_qk.tile([P, SC], F32, tag="psqk", name=f"psk{p}_{sc}")
        for i in range(NI):
            nc.tensor.matmul(ps_k[:], lhsT=wk[:, p, i, :],
                             rhs=xt[:, i, ssl], start=(i == 0), stop=(i == 7))
        nc.vector.tensor_add(kT_pair[p][:, ssl], ps_k[:],
                             bk_t[:, p:p + 1].to_broadcast((P, SC)))

    def v_sub(t):
        """v projection for seq tile t (all 8 heads) into v_aug[t]."""
        ps_v = ps_qk.tile([P, DG], F32, tag="psqk", name=f"psv{t}")
        for i in range(NI):
            nc.tensor.matmul(ps_v[:], lhsT=xt[:, i, P * t:P * (t + 1)],
                             rhs=wv[:, i, :], start=(i == 0), stop=(i == 7))
        va3 = v_aug[t].rearrange("p (h c) -> p h c", c=P)
        nc.vector.tensor_add(va3[:, :, DK:P],
                             ps_v[:].rearrange("p (h c) -> p h c", c=DK),
                             bv_t[:].rearrange("p (h c) -> p h c", c=DK))

    def attention_chunk(p, j, att, fillers):
        """Causal attention for head pair p, q chunk j; eager normalize."""
        avs = [ps_av.tile([P, SC], F32, tag="av", name=f"av{p}_{j}_{h}")
               for h in range(2)]
        nkb = 4 * (j + 1)
        kbs = list(range(4 * j, nkb)) + list(range(0, 4 * j))  # diag first
        pending = []
        issued = [0]

        def issue_av(item):
            kb, qlo, et = item
            et3 = et.rearrange("p (h w) -> p h w", w=SC)
            for h in range(2):
                hh = 2 * p + h
                va3 = v_aug[kb].rearrange("p (h c) -> p h c", c=P)
                nc.tensor.matmul(avs[h][:, qlo:], lhsT=va3[:, hh, :],
                                 rhs=et3[:, h, qlo:],
                                 start=(issued[0] == 0),
                                 stop=(issued[0] == nkb - 1))
            issued[0] += 1

        for n, kb in enumerate(kbs):
            r = kb - 4 * j  # >= 0 on diagonal blocks
            qlo = P * r if r >= 0 else 0
            ps_s = ps_sc.tile([P, 2 * SC], F32, tag="sc", name=f"pss{p}_{j}_{kb}")
            for h in range(2):
                hb = slice(DK * h, DK * (h + 1))
                nc.tensor.matmul(
                    ps_s[:, SC * h + qlo:SC * (h + 1)],
                    lhsT=kT_pair[p][hb, P * kb:P * (kb + 1)],
                    rhs=qT_pair[p][hb, SC * j + qlo:SC * (j + 1)],
                    start=True, stop=True)
            et = exp_pool.tile([P, 2 * SC], BF16, tag="exp", name=f"et{p}_{j}_{kb}")
            ps3 = ps_s.rearrange("p (h w) -> p h w", w=SC)
            et3 = et.rearrange("p (h w) -> p h w", w=SC)
            nc.scalar.activation(et3[:, :, qlo:], ps3[:, :, qlo:],
                                 mybir.ActivationFunctionType.Exp, scale=0.125)
            if r >= 0:
                nc.vector.tensor_mul(
                    et3[:, :, qlo:], et3[:, :, qlo:],
                    tri[:, 0:1, 0:SC - qlo].to_broadcast((P, 2, SC - qlo)))
            pending.append((kb, qlo, et))
            while len(pending) > 3:
                issue_av(pending.pop(0))
            if fillers:
                fillers.pop(0)()
        while pending:
            issue_av(pending.pop(0))

        # eager normalize straight from psum; denominator at partition 0
        for h in range(2):
            sums = small.tile([1, SC], F32, tag="sums", name=f"sums{p}_{j}_{h}")
            nc.vector.reciprocal_approx_fast(sums[0:1, :], avs[h][0:1, :])
            rb = small.tile([P, SC], F32, tag="rb", name=f"rb{p}_{j}_{h}")
            nc.gpsimd.partition_broadcast(rb[:], sums[0:1, :])
            nc.vector.tensor_mul(att[h][DK:P, SC * j:SC * (j + 1)],
                                 avs[h][DK:P, :], rb[DK:P, :])

    def agt_lhsT(i, qt):
        if qt < 12:
            return agt_a[i][:, P * qt:P * (qt + 1)]
        return agt_b[i][:, P * (qt - 12):P * (qt - 11)]

    def part_slice(qt):
        t = part_lo if qt < 8 else part_hi
        return t[:, qt % 8, :]

    def op_acc(qt, blocks, first):
        """Accumulate out-proj blocks into the fp16 partial for q-tile qt."""
        ps_o = ps_qk.tile([P, DG], F32, tag="psqk", name=f"pso{qt}_{blocks[0]}")
        for n, i in enumerate(blocks):
            nc.tensor.matmul(ps_o[:], lhsT=agt_lhsT(i, qt), rhs=wo[:, i, :],
                             start=(n == 0), stop=(n == len(blocks) - 1))
        t = part_slice(qt)
        nc.vector.tensor_add(t, ps_o[:], bo_t[:] if first else t)

    def op_final(qt):
        """Blocks 3,7 + partial -> out, DMA (alternating queues)."""
        ps_o = ps_qk.tile([P, DG], F32, tag="psqk", name=f"psof{qt}")
        for n, i in enumerate([3, 7]):
            nc.tensor.matmul(ps_o[:], lhsT=agt_lhsT(i, qt), rhs=wo[:, i, :],
                             start=(n == 0), stop=(n == 1))
        ot = out_pool.tile([P, DG], F32, tag="ot", name=f"ot{qt}")
        nc.vector.tensor_add(ot[:], ps_o[:], part_slice(qt))
        q = nc.sync if qt % 2 == 0 else nc.scalar
        q.dma_start(out[P * qt:P * (qt + 1), :], ot[:])

    def gather(in_ap, out_ap):
        nc.gpsimd.collective_compute(
            "AllGather", mybir.AluOpType.bypass, replica_groups=groups,
            ins=[in_ap.opt()], outs=[out_ap.opt()])

    # ---- prelude: pair-0 q/k, first half of pair-1 q/k, chunk-0 v ----
    for sc in range(NSC):
        qk_chunk(0, sc)
    qk_chunk(1, 0)
    qk_chunk(1, 1)
    for t in range(4):
        v_sub(t)

    # ---- per-chunk filler schedules ----
    def F(fn, *a):
        return lambda: fn(*a)

    # A-phase fillers sit >=1 full pair after their gather triggers, so the
    # ~10us collective latency (plus entry skew on gather0) never stalls PE.
    fillers = {
        0: [[F(v_sub, t) for t in range(4, 8)],
            [F(v_sub, t) for t in range(8, 12)],
            [F(v_sub, t) for t in range(12, 16)],
            [F(qk_chunk, 1, 2), F(qk_chunk, 1, 3)]],
        1: [[F(qk_chunk, 2, 0)],
            [F(qk_chunk, 2, 1)],
            [F(qk_chunk, 2, 2)],
            [F(qk_chunk, 2, 3)]],
        2: [[F(qk_chunk, 3, 0)],
            [F(qk_chunk, 3, 1)],
            [F(qk_chunk, 3, 2)] + [F(op_acc, qt, [0, 4], True) for qt in range(6)],
            [F(qk_chunk, 3, 3)] + [F(op_acc, qt, [0, 4], True) for qt in range(6, 16)]],
        3: [[],
            [F(op_acc, qt, [1, 5], False) for qt in range(8)],
            [F(op_acc, qt, [1, 5], False) for qt in range(8, 16)],
            [F(op_acc, qt, [2, 6], False) for qt in range(16)]],
    }

    # ---- attention pipeline ----
    for p in range(NPAIR):
        att = [att_pool.tile([P, S], BF16, tag=f"att{h}", name=f"att{p}_{h}")
               for h in range(2)]
        for j in range(NSC):
            ch = fillers[p][j]
            attention_chunk(p, j, att, ch)
            for f in ch:  # leftover fillers for this chunk
                f()
            ch.clear()
            if p == 3 and j == 2:
                # 3/4 gather for pair 3 (cols 0:1536)
                nc.gpsimd.dma_start(agin3a[0:DK, :], att[0][DK:P, 0:3 * SC])
                nc.gpsimd.dma_start(agin3a[DK:P, :], att[1][DK:P, 0:3 * SC])
                gather(agin3a[:], agout3a[:])
                for gs in range(2):
                    nc.sync.dma_start(agt_a[4 * gs + 3][:], agout3a[gs])
        if p < 3:
            nc.gpsimd.dma_start(agin[p][0:DK, :], att[0][DK:P, :])
            nc.gpsimd.dma_start(agin[p][DK:P, :], att[1][DK:P, :])
            gather(agin[p][:], agout[p][:])
            for gs in range(2):
                i = 4 * gs + p
                nc.sync.dma_start(agt_a[i][:], agout[p][gs][:, 0:3 * SC])
                nc.sync.dma_start(agt_b[i][:], agout[p][gs][:, 3 * SC:])
        else:
            # final 1/4 gather (cols 1536:2048)
            nc.gpsimd.dma_start(agin3b[0:DK, :], att[0][DK:P, 3 * SC:])
            nc.gpsimd.dma_start(agin3b[DK:P, :], att[1][DK:P, 3 * SC:])
            gather(agin3b[:], agout3b[:])
            for qt in range(12):       # B1: covered by the 3/4 gather
                op_final(qt)
            for gs in range(2):
                nc.gpsimd.dma_start(agt_b[4 * gs + 3][:], agout3b[gs])
            for qt in range(12, 16):   # B2: after the final gather
                op_final(qt)

    for cm in reversed(ctxs):
        cm.__exit__(None, None, None)


def _prep_in_maps(x, Wq, bq, Wk, bk, Wv, bv, Wo, bo):
    bf16 = ml_dtypes.bfloat16
    in_maps = []
    k_idx = np.arange(P)[:, None]
    q_idx = np.arange(SC)[None, :]
    mask = (q_idx >= k_idx).astype(bf16)

    def pair_layout(Wg):
        # [p, r, i, c] with W[128p+c, 128i+r]
        return np.ascontiguousarray(
            Wg.reshape(NPAIR, P, NI, P).transpose(0, 3, 2, 1)).astype(bf16)

    for c in range(8):
        b, g = divmod(c, 2)
        dsl = slice(g * DG, (g + 1) * DG)
        in_maps.append({
            "xT": np.ascontiguousarray(x[b].T).astype(bf16),
            "wqH": pair_layout(Wq[dsl]),
            "wkH": pair_layout(Wk[dsl]),
            "wvT": np.ascontiguousarray(Wv[dsl].T).astype(bf16),
            "woT": np.ascontiguousarray(Wo[dsl].T).astype(bf16),
            "bq": np.ascontiguousarray(bq[dsl].reshape(NPAIR, P).T.astype(np.float32)),
            "bk": np.ascontiguousarray(bk[dsl].reshape(NPAIR, P).T.astype(np.float32)),
            "bv_bc": np.broadcast_to(bv[dsl].astype(np.float32), (P, DG)).copy(),
            "bo_bc": np.broadcast_to(bo[dsl].astype(np.float32), (P, DG)).copy(),
            "masks": mask,
        })
    return in_maps


def kernel(x, Wq, bq, Wk, bk, Wv, bv, Wo, bo, _trace=False, _trace_kwargs=None):
    x, Wq, bq, Wk, bk = map(np.asarray, (x, Wq, bq, Wk, bk))
    Wv, bv, Wo, bo = map(np.asarray, (Wv, bv, Wo, bo))
    if "nc" not in _cache:
        _cache["nc"] = _build()
    nc = _cache["nc"]
    in_maps = _prep_in_maps(x, Wq, bq, Wk, bk, Wv, bv, Wo, bo)
    res = bass_utils.run_bass_kernel_spmd(
        nc, in_maps, core_ids=list(range(8)), trace=_trace,
        **(_trace_kwargs or {}))
    _cache["last_result"] = res
    out = np.empty((B, S, D), dtype=np.float32)
    for c in range(8):
        b, g = divmod(c, 2)
        out[b, :, g * DG:(g + 1) * DG] = res.results[c]["out"]
    return out


# revision 38
# speedup vs baseline: 1.0789x; 1.0176x over previous
"""Causal multi-head attention on 8 TRN2 NeuronCores — v2.

Sharding: core c = (batch b=c//2, head-group g=c%2). Each core computes QKV
projections + causal attention for its 8 heads over the full sequence of its
batch; pairwise AllGathers exchange attention outputs; each core runs the
output projection for its half of the output columns (per-core host data
keeps the program uniform across cores).

v2 scheduling changes vs v1 (same math, bf16 everywhere):
  - input DMAs split across the Sync and Scalar queues, ordered by first
    use; q/k weights in per-pair host layout so each pair loads with 2KB
    lines just before it is needed.
  - v_aug ones/zero pattern comes from memsets instead of a DMA'd pattern
    (ones col 0 -> softmax denominator in psum row 0, values rows 64:128;
    reciprocal_approx_fast and partition_broadcast require base 0).
  - normalization is eager at the end of each chunk and reads the av psum
    directly (no bounce copies).
  - output projection is spread across the attention pipeline as PE
    fillers: A1{0,4}@p1, A2{1,5}@p2, A3{2,6}@p3, then B{3,7} per q-tile.
    Pair 3's attention is gathered in two pieces (cols 0:1536 after j=2,
    cols 1536:2048 after j=3) so only 4 q-tiles of work remain after the
    last (small) AllGather.
  - one AllGather per pair (absorbs inter-core skew earlier).
"""

import numpy as np
import ml_dtypes

import concourse.bass as bass
import concourse.mybir as mybir
import concourse.tile as tile
from concourse import bacc
from concourse import bass_utils

BF16 = mybir.dt.bfloat16
FP16 = mybir.dt.float16
F32 = mybir.dt.float32

B, S, D = 4, 2048, 1024
H, DK = 16, 64
HPG = 8          # heads per group (per core)
DG = HPG * DK    # 512, d-range per core
NPAIR = 4        # head pairs per core
SC = 512         # sequence chunk (matmul free dim)
NSC = S // SC    # 4
KB = 128         # key block
NKB = S // KB    # 16
P = 128
NI = D // P      # 8

_cache = {}


def _build():
    nc = bacc.Bacc("TRN2", target_bir_lowering=False, debug=False, num_devices=8)

    xT = nc.dram_tensor("xT", [D, S], BF16, kind="ExternalInput")
    wqH = nc.dram_tensor("wqH", [NPAIR, P, NI, P], BF16, kind="ExternalInput")
    wkH = nc.dram_tensor("wkH", [NPAIR, P, NI, P], BF16, kind="ExternalInput")
    wvT = nc.dram_tensor("wvT", [D, DG], BF16, kind="ExternalInput")
    woT = nc.dram_tensor("woT", [D, DG], BF16, kind="ExternalInput")
    bq = nc.dram_tensor("bq", [P, NPAIR], F32, kind="ExternalInput")
    bk = nc.dram_tensor("bk", [P, NPAIR], F32, kind="ExternalInput")
    bv_bc = nc.dram_tensor("bv_bc", [P, DG], F32, kind="ExternalInput")
    bo_bc = nc.dram_tensor("bo_bc", [P, DG], F32, kind="ExternalInput")
    masks = nc.dram_tensor("masks", [P, SC], BF16, kind="ExternalInput")
    out = nc.dram_tensor("out", [S, DG], F32, kind="ExternalOutput")

    with tile.TileContext(nc) as tc:
        _emit(nc, tc, xT, wqH, wkH, wvT, woT, bq, bk, bv_bc, bo_bc, masks, out)
    nc.compile()
    return nc


def _emit(nc, tc, xT, wqH, wkH, wvT, woT, bq, bk, bv_bc, bo_bc, masks, out):
    ctxs = []

    def pool(name, bufs, space="SBUF"):
        cm = tc.tile_pool(name=name, bufs=bufs, space=space)
        p = cm.__enter__()
        ctxs.append(cm)
        return p

    const = pool("const", 1)
    dram = pool("dram", 1, space="DRAM")
    qk_pool = pool("qk", 2)
    att_pool = pool("att", 2)
    exp_pool = pool("exp", 4)
    small = pool("small", 3)
    out_pool = pool("outp", 3)
    ps_qk = pool("ps_qk", 2, space="PSUM")
    ps_sc = pool("ps_sc", 2, space="PSUM")
    ps_av = pool("ps_av", 2, space="PSUM")

    # ---- constant tiles ----
    xt = const.tile([P, NI, S], BF16, name="xt")
    wq = const.tile([P, NPAIR, NI, P], BF16, name="wq")
    wk = const.tile([P, NPAIR, NI, P], BF16, name="wk")
    wv = const.tile([P, NI, DG], BF16, name="wv")
    wo = const.tile([P, NI, DG], BF16, name="wo")
    bq_t = const.tile([P, NPAIR], F32, name="bq_t")
    bk_t = const.tile([P, NPAIR], F32, name="bk_t")
    bv_t = const.tile([P, DG], F32, name="bv_t")
    bo_t = const.tile([P, DG], F32, name="bo_t")
    # single lower-triangle mask; mask for diag offset r is tri[:, 0, 0:SC-128r]
    tri = const.tile([P, 1, SC], BF16, name="tri")
    # v_aug[t]: [128, 8, 128]; per head h: col 0 = ones (softmax denominator
    # row in psum partition 0), cols 1:64 = zeros (partition alignment pad),
    # cols 64:128 = v.
    v_aug = [const.tile([P, HPG * P], BF16, name=f"va{t}", tag=f"va{t}")
             for t in range(NKB)]

    # ---- input DMA preloads, ordered by first use, split across queues ----
    # sync queue: first matmul's inputs first, then the rest of chunk 0
    nc.sync.dma_start(xt[:, 0, 0:SC], xT[0:P, 0:SC])
    nc.sync.dma_start(wq[:, 0], wqH[0])
    nc.sync.dma_start(wk[:, 0], wkH[0])
    for i in range(1, NI):
        nc.sync.dma_start(xt[:, i, 0:SC], xT[P * i:P * (i + 1), 0:SC])
    # gpsimd: q/k biases (tiny, needed by the first DVE adds)
    nc.gpsimd.dma_start(bq_t[:], bq[:])
    nc.gpsimd.dma_start(bk_t[:], bk[:])
    # scalar queue: remaining weights (scalar engine is idle until first exp)
    nc.scalar.dma_start(wq[:, 1], wqH[1])
    nc.scalar.dma_start(wk[:, 1], wkH[1])
    for i in range(NI):
        nc.scalar.dma_start(wv[:, i, :], wvT[P * i:P * (i + 1), :])
    nc.scalar.dma_start(bv_t[:], bv_bc[:])
    nc.scalar.dma_start(tri[:, 0, :], masks[:])
    for p in range(2, NPAIR):
        nc.scalar.dma_start(wq[:, p], wqH[p])
        nc.scalar.dma_start(wk[:, p], wkH[p])
    # sync queue: rest of x, then the out-projection weights
    for c in range(1, NSC):
        for i in range(NI):
            nc.sync.dma_start(xt[:, i, SC * c:SC * (c + 1)],
                              xT[P * i:P * (i + 1), SC * c:SC * (c + 1)])
    for i in range(NI):
        nc.sync.dma_start(wo[:, i, :], woT[P * i:P * (i + 1), :])
    nc.sync.dma_start(bo_t[:], bo_bc[:])
    # ones/zeros columns of v_aug (gpsimd, cheap)
    for t in range(NKB):
        va3 = v_aug[t].rearrange("p (h c) -> p h c", c=P)
        nc.gpsimd.memset(va3[:, :, 0:DK], 0.0)
        nc.gpsimd.memset(va3[:, :, 0:1], 1.0)

    # DRAM bounce buffers for the AllGathers
    agin = [dram.tile([P, S], BF16, name=f"agin{p}") for p in range(3)]
    agout = [dram.tile([2, P, S], BF16, name=f"agout{p}") for p in range(3)]
    agin3a = dram.tile([P, 3 * SC], BF16, name="agin3a")
    agin3b = dram.tile([P, SC], BF16, name="agin3b")
    agout3a = dram.tile([2, P, 3 * SC], BF16, name="agout3a")
    agout3b = dram.tile([2, P, SC], BF16, name="agout3b")

    groups = [[0, 1], [2, 3], [4, 5], [6, 7]]

    qT_pair = [qk_pool.tile([P, S], BF16, tag="qT", name=f"qTp{pp}")
               for pp in range(NPAIR)]
    kT_pair = [qk_pool.tile([P, S], BF16, tag="kT", name=f"kTp{pp}")
               for pp in range(NPAIR)]

    # gathered attention outputs (out-proj lhsT), split 1536/512 in q to
    # match pair 3's two gathers
    agt_a = [const.tile([P, 3 * SC], BF16, name=f"agta{i}", tag=f"agta{i}")
             for i in range(NI)]
    agt_b = [const.tile([P, SC], BF16, name=f"agtb{i}", tag=f"agtb{i}")
             for i in range(NI)]
    # out-proj partials (blocks 0,1,2,4,5,6 accumulated via fp16)
    part_lo = const.tile([P, NI, SC], FP16, name="part_lo")
    part_hi = const.tile([P, NI, SC], FP16, name="part_hi")

    def qk_chunk(p, sc):
        """q/k projections for pair p, seq chunk sc."""
        ssl = slice(SC * sc, SC * (sc + 1))
        ps_q = ps_qk.tile([P, SC], F32, tag="psqk", name=f"psq{p}_{sc}")
        for i in range(NI):
            nc.tensor.matmul(ps_q[:], lhsT=wq[:, p, i, :],
                             rhs=xt[:, i, ssl], start=(i == 0), stop=(i == 7))
        nc.vector.tensor_add(qT_pair[p][:, ssl], ps_q[:],
                             bq_t[:, p:p + 1].to_broadcast((P, SC)))
        ps_k = ps_qk.tile([P, SC], F32, tag="psqk", name=f"psk{p}_{sc}")
        for i in range(NI):
            nc.tensor.matmul(ps_k[:], lhsT=wk[:, p, i, :],
                             rhs=xt[:, i, ssl], start=(i == 0), stop=(i == 7))
        nc.vector.tensor_add(kT_pair[p][:, ssl], ps_k[:],
                             bk_t[:, p:p + 1].to_broadcast((P, SC)))

    def v_sub(t):
        """v projection for seq tile t (all 8 heads) into v_aug[t]."""
        ps_v = ps_qk.tile([P, DG], F32, tag="psqk", name=f"psv{t}")
        for i in range(NI):
            nc.tensor.matmul(ps_v[:], lhsT=xt[:, i, P * t:P * (t + 1)],
                             rhs=wv[:, i, :], start=(i == 0), stop=(i == 7))
        va3 = v_aug[t].rearrange("p (h c) -> p h c", c=P)
        nc.vector.tensor_add(va3[:, :, DK:P],
                             ps_v[:].rearrange("p (h c) -> p h c", c=DK),
                             bv_t[:].rearrange("p (h c) -> p h c", c=DK))

    def attention_chunk(p, j, att, fillers):
        """Causal attention for head pair p, q chunk j; eager normalize."""
        avs = [ps_av.tile([P, SC], F32, tag="av", name=f"av{p}_{j}_{h}")
               for h in range(2)]
        nkb = 4 * (j + 1)
        kbs = list(range(4 * j, nkb)) + list(range(0, 4 * j))  # diag first
        pending = []
        issued = [0]

        def issue_av(item):
            kb, qlo, et = item
            et3 = et.rearrange("p (h w) -> p h w", w=SC)
            for h in range(2):
                hh = 2 * p + h
                va3 = v_aug[kb].rearrange("p (h c) -> p h c", c=P)
                nc.tensor.matmul(avs[h][:, qlo:], lhsT=va3[:, hh, :],
                                 rhs=et3[:, h, qlo:],
                                 start=(issued[0] == 0),
                                 stop=(issued[0] == nkb - 1))
            issued[0] += 1

        for n, kb in enumerate(kbs):
            r = kb - 4 * j  # >= 0 on diagonal blocks
            qlo = P * r if r >= 0 else 0
            ps_s = ps_sc.tile([P, 2 * SC], F32, tag="sc", name=f"pss{p}_{j}_{kb}")
            for h in range(2):
                hb = slice(DK * h, DK * (h + 1))
                nc.tensor.matmul(
                    ps_s[:, SC * h + qlo:SC * (h + 1)],
                    lhsT=kT_pair[p][hb, P * kb:P * (kb + 1)],
                    rhs=qT_pair[p][hb, SC * j + qlo:SC * (j + 1)],
                    start=True, stop=True)
            et = exp_pool.tile([P, 2 * SC], BF16, tag="exp", name=f"et{p}_{j}_{kb}")
            ps3 = ps_s.rearrange("p (h w) -> p h w", w=SC)
            et3 = et.rearrange("p (h w) -> p h w", w=SC)
            nc.scalar.activation(et3[:, :, qlo:], ps3[:, :, qlo:],
                                 mybir.ActivationFunctionType.Exp, scale=0.125)
            if r >= 0:
                nc.vector.tensor_mul(
                    et3[:, :, qlo:], et3[:, :, qlo:],
                    tri[:, 0:1, 0:SC - qlo].to_broadcast((P, 2, SC - qlo)))
            pending.append((kb, qlo, et))
            while len(pending) > 2:
                issue_av(pending.pop(0))
            if fillers:
                fillers.pop(0)()
        while pending:
            issue_av(pending.pop(0))

        # eager normalize straight from psum; denominator at partition 0
        for h in range(2):
            sums = small.tile([1, SC], F32, tag="sums", name=f"sums{p}_{j}_{h}")
            nc.vector.reciprocal_approx_fast(sums[0:1, :], avs[h][0:1, :])
            rb = small.tile([P, SC], F32, tag="rb", name=f"rb{p}_{j}_{h}")
            nc.gpsimd.partition_broadcast(rb[:], sums[0:1, :])
            nc.vector.tensor_mul(att[h][DK:P, SC * j:SC * (j + 1)],
                                 avs[h][DK:P, :], rb[DK:P, :])

    def agt_lhsT(i, qt):
        if qt < 12:
            return agt_a[i][:, P * qt:P * (qt + 1)]
        return agt_b[i][:, P * (qt - 12):P * (qt - 11)]

    def part_slice(qt):
        t = part_lo if qt < 8 else part_hi
        return t[:, qt % 8, :]

    def op_acc(qt, blocks, first):
        """Accumulate out-proj blocks into the fp16 partial for q-tile qt."""
        ps_o = ps_qk.tile([P, DG], F32, tag="psqk", name=f"pso{qt}_{blocks[0]}")
        for n, i in enumerate(blocks):
            nc.tensor.matmul(ps_o[:], lhsT=agt_lhsT(i, qt), rhs=wo[:, i, :],
                             start=(n == 0), stop=(n == len(blocks) - 1))
        t = part_slice(qt)
        nc.vector.tensor_add(t, ps_o[:], bo_t[:] if first else t)

    def op_final(qt):
        """Blocks 3,7 + partial -> out, DMA (alternating queues)."""
        ps_o = ps_qk.tile([P, DG], F32, tag="psqk", name=f"psof{qt}")
        for n, i in enumerate([3, 7]):
            nc.tensor.matmul(ps_o[:], lhsT=agt_lhsT(i, qt), rhs=wo[:, i, :],
                             start=(n == 0), stop=(n == 1))
        ot = out_pool.tile([P, DG], F32, tag="ot", name=f"ot{qt}")
        nc.vector.tensor_add(ot[:], ps_o[:], part_slice(qt))
        q = nc.sync if qt % 2 == 0 else nc.scalar
        q.dma_start(out[P * qt:P * (qt + 1), :], ot[:])

    def gather(in_ap, out_ap):
        nc.gpsimd.collective_compute(
            "AllGather", mybir.AluOpType.bypass, replica_groups=groups,
            ins=[in_ap.opt()], outs=[out_ap.opt()])

    # ---- prelude: pair-0 q/k, first half of pair-1 q/k, chunk-0 v ----
    for sc in range(NSC):
        qk_chunk(0, sc)
    qk_chunk(1, 0)
    qk_chunk(1, 1)
    for t in range(4):
        v_sub(t)

    # ---- per-chunk filler schedules ----
    def F(fn, *a):
        return lambda: fn(*a)

    # A-phase fillers sit >=1 full pair after their gather triggers, so the
    # ~10us collective latency (plus entry skew on gather0) never stalls PE.
    fillers = {
        0: [[F(v_sub, t) for t in range(4, 8)],
            [F(v_sub, t) for t in range(8, 12)],
            [F(v_sub, t) for t in range(12, 16)],
            [F(qk_chunk, 1, 2), F(qk_chunk, 1, 3)]],
        1: [[F(qk_chunk, 2, 0)],
            [F(qk_chunk, 2, 1)],
            [F(qk_chunk, 2, 2)],
            [F(qk_chunk, 2, 3)]],
        2: [[F(qk_chunk, 3, 0)],
            [F(qk_chunk, 3, 1)],
            [F(qk_chunk, 3, 2)] + [F(op_acc, qt, [0, 4], True) for qt in range(6)],
            [F(qk_chunk, 3, 3)] + [F(op_acc, qt, [0, 4], True) for qt in range(6, 16)]],
        3: [[],
            [F(op_acc, qt, [1, 5], False) for qt in range(8)],
            [F(op_acc, qt, [1, 5], False) for qt in range(8, 16)],
            [F(op_acc, qt, [2, 6], False) for qt in range(12)]],
    }

    # ---- attention pipeline ----
    for p in range(NPAIR):
        att = [att_pool.tile([P, S], BF16, tag=f"att{h}", name=f"att{p}_{h}")
               for h in range(2)]
        for j in range(NSC):
            ch = fillers[p][j]
            attention_chunk(p, j, att, ch)
            for f in ch:  # leftover fillers for this chunk
                f()
            ch.clear()
            if p == 3 and j == 2:
                # 3/4 gather for pair 3 (cols 0:1536)
                nc.sync.dma_start(agin3a[0:DK, :], att[0][DK:P, 0:3 * SC])
                nc.sync.dma_start(agin3a[DK:P, :], att[1][DK:P, 0:3 * SC])
                gather(agin3a[:], agout3a[:])
                for gs in range(2):
                    nc.sync.dma_start(agt_a[4 * gs + 3][:], agout3a[gs])
        if p < 3:
            nc.sync.dma_start(agin[p][0:DK, :], att[0][DK:P, :])
            nc.sync.dma_start(agin[p][DK:P, :], att[1][DK:P, :])
            gather(agin[p][:], agout[p][:])
            for gs in range(2):
                i = 4 * gs + p
                nc.sync.dma_start(agt_a[i][:], agout[p][gs][:, 0:3 * SC])
                nc.sync.dma_start(agt_b[i][:], agout[p][gs][:, 3 * SC:])
        else:
            # final 1/4 gather (cols 1536:2048)
            nc.sync.dma_start(agin3b[0:DK, :], att[0][DK:P, 3 * SC:])
            nc.sync.dma_start(agin3b[DK:P, :], att[1][DK:P, 3 * SC:])
            gather(agin3b[:], agout3b[:])
            for qt in range(12, 16):   # A3 for the last q-tiles: covers the
                op_acc(qt, [2, 6], False)  # gather latency with PE work
            for qt in range(12):       # B1: covered by the 3/4 gather
                op_final(qt)
            for gs in range(2):
                nc.gpsimd.dma_start(agt_b[4 * gs + 3][:], agout3b[gs])
            for qt in range(12, 16):   # B2: after the final gather
                op_final(qt)

    for cm in reversed(ctxs):
        cm.__exit__(None, None, None)


def _prep_in_maps(x, Wq, bq, Wk, bk, Wv, bv, Wo, bo):
    bf16 = ml_dtypes.bfloat16
    in_maps = []
    k_idx = np.arange(P)[:, None]
    q_idx = np.arange(SC)[None, :]
    mask = (q_idx >= k_idx).astype(bf16)

    def pair_layout(Wg):
        # [p, r, i, c] with W[128p+c, 128i+r]
        return np.ascontiguousarray(
            Wg.reshape(NPAIR, P, NI, P).transpose(0, 3, 2, 1)).astype(bf16)

    for c in range(8):
        b, g = divmod(c, 2)
        dsl = slice(g * DG, (g + 1) * DG)
        in_maps.append({
            "xT": np.ascontiguousarray(x[b].T).astype(bf16),
            "wqH": pair_layout(Wq[dsl]),
            "wkH": pair_layout(Wk[dsl]),
            "wvT": np.ascontiguousarray(Wv[dsl].T).astype(bf16),
            "woT": np.ascontiguousarray(Wo[dsl].T).astype(bf16),
            "bq": np.ascontiguousarray(bq[dsl].reshape(NPAIR, P).T.astype(np.float32)),
            "bk": np.ascontiguousarray(bk[dsl].reshape(NPAIR, P).T.astype(np.float32)),
            "bv_bc": np.broadcast_to(bv[dsl].astype(np.float32), (P, DG)).copy(),
            "bo_bc": np.broadcast_to(bo[dsl].astype(np.float32), (P, DG)).copy(),
            "masks": mask,
        })
    return in_maps


def kernel(x, Wq, bq, Wk, bk, Wv, bv, Wo, bo, _trace=False, _trace_kwargs=None):
    x, Wq, bq, Wk, bk = map(np.asarray, (x, Wq, bq, Wk, bk))
    Wv, bv, Wo, bo = map(np.asarray, (Wv, bv, Wo, bo))
    if "nc" not in _cache:
        _cache["nc"] = _build()
    nc = _cache["nc"]
    in_maps = _prep_in_maps(x, Wq, bq, Wk, bk, Wv, bv, Wo, bo)
    res = bass_utils.run_bass_kernel_spmd(
        nc, in_maps, core_ids=list(range(8)), trace=_trace,
        **(_trace_kwargs or {}))
    _cache["last_result"] = res
    out = np.empty((B, S, D), dtype=np.float32)
    for c in range(8):
        b, g = divmod(c, 2)
        out[b, :, g * DG:(g + 1) * DG] = res.results[c]["out"]
    return out
